# revision 12
# baseline (speedup 1.0000x reference)
"""Trainium2 Bass kernel for a ViT-style transformer block (nn_Block_11132555231612).

Data-parallel over batch across 8 NeuronCores (2 sequences of 1024 tokens per
core). fp8e4m3 DoubleRow matmuls (0.5 cyc/row, 256-deep contraction) carry
QKV / AV / proj / fc1 / fc2; scores stay bf16. The residual stream is held at
32x scale (x scaled on host, weights scaled to match, output unscaled on
host) so fp8 weight scales fold away with no device fixups; the softmax
ones-column trick (column value = 32) cancels the V scale. fc1/fc2 weights
use hi+lo double-fp8 planes accumulated in PSUM for near-bf16 weight
precision.

The two sequences are pipelined as interleaved emission streams (engines
execute their queues in order, so overlap requires interleaving): seq-1
LayerNorm runs under seq-0 attention, and seq-0's fc2 (pure PE+DVE) runs
under seq-1's attention, whose critical resource is the scalar engine's
softmax exp. fc1+gelu run in the inter-attention valley where the scalar
engine is free. PSUM: A=[P,2,512]x2 (scores & fc1), B=[P,512]x2 (AV
accumulators), C=[P,512-slot]x2 (all other matmul outputs) = exactly 8 banks.
"""

import os
import sys

sys.path.insert(0, "/opt/trn_rl_repo")

import numpy as np
import ml_dtypes

import concourse.bass as bass
import concourse.mybir as mybir
import concourse.tile as tile
from concourse import bacc
from concourse.bass_utils import run_bass_kernel_spmd
from concourse.masks import make_identity
from contextlib import ExitStack

F32 = mybir.dt.float32
BF16 = mybir.dt.bfloat16
FP8 = mybir.dt.float8e4
NP8 = ml_dtypes.float8_e4m3
AF = mybir.ActivationFunctionType
DR = mybir.MatmulPerfMode.DoubleRow
ALU = mybir.AluOpType

P = 128
B_PER_CORE = 2
SEQ = 1024
T = B_PER_CORE * SEQ          # 2048 tokens per core
C = 768
H = 12
HD = 64
HID = 3072
KS = C // P                   # 6
HS = HID // P                 # 24
NT = T // P                   # 16 token tiles
NB = NT // B_PER_CORE         # 8 token tiles per sequence
EPS = 1e-5
SCALE = HD ** -0.5            # 0.125
S = 32.0                      # residual / weight scale
VP = 80                       # padded V row (65 used): 16B dual-fp8 ldweights

_CACHED_NC = None


class TileKernel:
    b1_zero = False
    bv_zero = False
    bproj_zero = False
    b2_zero = False
    bqk_zero = False
    w1x2 = True
    w2x2 = True

    def __init__(self, nc):
        self.nc = nc
        self.stack = ExitStack()
        self.tc = None

    def __enter__(self):
        self.tc = self.stack.enter_context(tile.TileContext(self.nc))
        return self

    def __exit__(self, *exc):
        return self.stack.__exit__(*exc)

    def ln_tile(self, xt, dst, dst_col):
        """LN of one token-major tile xt [P, C] -> feature-major columns
        dst[:, :, dst_col:dst_col+P]. Stats on DVE, apply on GPSIMD,
        transpose on PE (bf16; hw rejects fp8 transposes), psum->sbuf copy
        converts to dst dtype."""
        nc, work, psC = self.nc, self.work, self.psC
        st = work.tile([P, 3, 6], F32, tag="bnstats")
        xg = xt.rearrange("p (s d) -> p s d", s=3)
        for s in range(3):
            nc.vector.bn_stats(st[:, s, :], xg[:, s, :])
        mv = work.tile([P, 2], F32, tag="mv")
        nc.vector.bn_aggr(mv[:], st[:])
        sdv = work.tile([P, 1], F32, tag="sdv")
        nc.scalar.activation(sdv[:], mv[:, 1:2], AF.Sqrt, bias=self.eps_t[:])
        rstd = work.tile([P, 1], F32, tag="rstd")
        nc.vector.reciprocal(rstd[:], sdv[:])
        nmu = work.tile([P, 1], F32, tag="nmu")
        nc.vector.tensor_scalar_mul(nmu[:], mv[:, 0:1], -1.0)
        xn = work.tile([P, C], BF16, tag="xn")
        nc.gpsimd.tensor_scalar(xn[:], xt, nmu[:], rstd[:],
                                op0=ALU.add, op1=ALU.mult)
        for c in range(KS):
            pt = psC.tile([P, P], BF16, tag="C", name="pt")
            nc.tensor.transpose(pt[:], xn[:, c * P:(c + 1) * P],
                                self.ident16[:])
            nc.vector.tensor_copy(dst[:, c, dst_col:dst_col + P], pt[:])

    def v_unit(self, t, xnT_b, V_b):
        """V for one token tile (t local to the sequence) into V_b, via two
        C-ring psum chunks (heads 0-7, heads 8-11)."""
        nc, psC = self.nc, self.psC
        for (n0, nh) in ((0, 8), (512, 4)):
            psv = psC.tile([P, nh * HD], F32, tag="C", name="psv")
            for c2 in range(3):
                nc.tensor.matmul(
                    psv[:],
                    xnT_b[:, c2 * 2:c2 * 2 + 2, t * P:(t + 1) * P],
                    self.wqkv_sb[:, c2 * 2:c2 * 2 + 2,
                                 2 * C + n0:2 * C + n0 + nh * HD],
                    start=(c2 == 0), stop=(c2 == 2), perf_mode=DR)
            h0 = n0 // HD
            if self.bv_zero:
                nc.scalar.copy(
                    V_b[:, t, h0:h0 + nh, 0:HD],
                    psv[:].rearrange("p (h d) -> p h d", h=nh))
            else:
                nc.vector.tensor_add(
                    V_b[:, t, h0:h0 + nh, 0:HD],
                    psv[:].rearrange("p (h d) -> p h d", h=nh),
                    self.bv_bc[:, h0 * HD:(h0 + nh) * HD].rearrange(
                        "p (h d) -> p h d", h=nh))

    def qk_unit(self, oct, nch, xnT_b, qkT_b):
        """One q/k slice (oct) for one 512-token chunk of a sequence."""
        nc, psC = self.nc, self.psC
        ps = psC.tile([P, 512], F32, tag="C", name="ps")
        for c2 in range(3):
            nc.tensor.matmul(
                ps[:],
                self.wqkv_sb[:, c2 * 2:c2 * 2 + 2, oct * P:(oct + 1) * P],
                xnT_b[:, c2 * 2:c2 * 2 + 2, nch * 512:(nch + 1) * 512],
                start=(c2 == 0), stop=(c2 == 2), perf_mode=DR)
        if self.bqk_zero:
            nc.scalar.copy(qkT_b[:, oct, nch * 512:(nch + 1) * 512], ps[:])
        else:
            nc.vector.tensor_scalar_add(
                qkT_b[:, oct, nch * 512:(nch + 1) * 512], ps[:],
                self.bqkv_sb[:, oct:oct + 1])

    def attn_unit(self, h, qc, qkT_b, V_b, oT_b):
        """Scores+softmax+AV+normalize for one (head, 512-query chunk)."""
        nc, psA, psB, awork = self.nc, self.psA, self.psB, self.awork
        po = (h % 2) * 64
        oq, ok = h // 2, 6 + h // 2
        qs = qc * 512
        pso = psB.tile([P, 512], F32, tag="B", name="pso")
        for kt2 in range(4):
            pss = psA.tile([P, 2, 512], F32, tag="A", name="pss")
            for j in range(2):
                ko = (2 * kt2 + j) * P
                nc.tensor.matmul(
                    pss[:, j, :],
                    qkT_b[po:po + HD, ok, ko:ko + P],
                    qkT_b[po:po + HD, oq, qs:qs + 512],
                    start=True, stop=True)
            pr = awork.tile([P, 2, 512], FP8, tag="probs")
            nc.scalar.activation(pr[:], pss[:], AF.Exp, scale=SCALE / (S * S))
            nc.tensor.matmul(
                pso[0:HD + 1, :],
                V_b[:, 2 * kt2:2 * kt2 + 2, h, 0:HD + 1],
                pr[:], start=(kt2 == 0), stop=(kt2 == 3), perf_mode=DR)
        rc = awork.tile([P, 512], BF16, tag="recip")
        with nc.allow_low_precision(reason="softmax denom reciprocal in bf16"):
            nc.vector.reciprocal(rc[HD:HD + 1, :], pso[HD:HD + 1, :])
        rc0 = awork.tile([1, 512], BF16, tag="rc0")
        nc.sync.dma_start(rc0[:], rc[HD:HD + 1, :])
        rbc = awork.tile([HD, 512], BF16, tag="rbc")
        nc.gpsimd.partition_broadcast(rbc[:], rc0[0:1, :], channels=HD)
        if h % 2 == 0:
            nc.vector.tensor_mul(oT_b[0:HD, h // 2, qs:qs + 512],
                                 pso[0:HD, :], rbc[:])
        else:
            osc = awork.tile([HD, 512], FP8, tag="osc")
            nc.vector.tensor_mul(osc[:], pso[0:HD, :], rbc[:])
            nc.sync.dma_start(oT_b[64:128, h // 2, qs:qs + 512], osc[:])

    def proj_tile(self, tg, lt, oT_b):
        """Attention projection + residual add for global tile tg (lt local
        within the sequence), then LN2 into xnT2 column."""
        nc, psC = self.nc, self.psC
        for (n0, nsz) in ((0, 512), (512, 256)):
            psp = psC.tile([P, nsz], F32, tag="C", name="psp")
            for c2 in range(3):
                nc.tensor.matmul(
                    psp[:],
                    oT_b[:, c2 * 2:c2 * 2 + 2, lt * P:(lt + 1) * P],
                    self.wproj_sb[:, c2 * 2:c2 * 2 + 2, n0:n0 + nsz],
                    start=(c2 == 0), stop=(c2 == 2), perf_mode=DR)
            nc.vector.tensor_add(x_part(self.x_sb, tg, n0, nsz),
                                 x_part(self.x_sb, tg, n0, nsz), psp[:])
            if not self.bproj_zero:
                nc.vector.tensor_add(
                    x_part(self.x_sb, tg, n0, nsz),
                    x_part(self.x_sb, tg, n0, nsz),
                    self.bproj_bc[:, n0:n0 + nsz])

    def fc1_unit(self, hp, tq_col, xnT2_b, hT_t):
        """fc1 + gelu for one hidden-feature pair over 512 tokens.
        tq_col: 0 or 512 (column offset within the sequence's xnT2)."""
        nc, psA = self.nc, self.psA
        n1 = 3 * len(self.w1_planes)
        ps1 = psA.tile([P, 2, 512], F32, tag="A", name="ps1")
        for j in range(2):
            i = 0
            for w1p in self.w1_planes:
                for c3 in range(3):
                    nc.tensor.matmul(
                        ps1[:, j, :],
                        w1p[:, c3 * 2:c3 * 2 + 2,
                            (hp * 2 + j) * P:(hp * 2 + j + 1) * P],
                        xnT2_b[:, c3 * 2:c3 * 2 + 2, tq_col:tq_col + 512],
                        start=(i == 0), stop=(i == n1 - 1), perf_mode=DR)
                    i += 1
        if self.b1_zero:
            nc.scalar.activation(
                hT_t[:, hp * 2:hp * 2 + 2, :].rearrange("p a b -> p (a b)"),
                ps1[:].rearrange("p a b -> p (a b)"), AF.Gelu, scale=1.0 / S)
        else:
            for j in range(2):
                nc.scalar.activation(
                    hT_t[:, hp * 2 + j, :], ps1[:, j, :], AF.Gelu,
                    bias=self.b1_sb[:, hp * 2 + j:hp * 2 + j + 1],
                    scale=1.0 / S)

    def fc2_unit(self, tg, tt, hT_t):
        """fc2 + residual add for global token tile tg (tt: 0..3 within the
        hT tile)."""
        nc, psC = self.nc, self.psC
        n2 = 12 * len(self.w2_planes)
        for (n0, nsz) in ((0, 512), (512, 256)):
            ps2 = psC.tile([P, nsz], F32, tag="C", name="ps2")
            i = 0
            for hp in range(12):
                for w2p in self.w2_planes:
                    nc.tensor.matmul(
                        ps2[:],
                        hT_t[:, hp * 2:hp * 2 + 2, tt * P:(tt + 1) * P],
                        w2p[:, hp * 2:hp * 2 + 2, n0:n0 + nsz],
                        start=(i == 0), stop=(i == n2 - 1), perf_mode=DR)
                    i += 1
            nc.vector.tensor_add(x_part(self.x_sb, tg, n0, nsz),
                                 x_part(self.x_sb, tg, n0, nsz), ps2[:])
            if not self.b2_zero:
                nc.vector.tensor_add(x_part(self.x_sb, tg, n0, nsz),
                                     x_part(self.x_sb, tg, n0, nsz),
                                     self.b2_bc[:, n0:n0 + nsz])

    def run(self, x_d, out_d, wqkv_d, bqkv_d, bv_d, wproj_d, bproj_d,
            w1h_d, w1l_d, b1_d, w2h_d, w2l_d, b2_d):
        nc, tc, S_ = self.nc, self.tc, self.stack
        const = S_.enter_context(tc.tile_pool(name="const", bufs=1))
        xpool = S_.enter_context(tc.tile_pool(name="xres", bufs=1))
        self.work = S_.enter_context(tc.tile_pool(name="work", bufs=2))
        self.psA = S_.enter_context(tc.tile_pool(name="psA", bufs=2,
                                                 space="PSUM"))
        self.psB = S_.enter_context(tc.tile_pool(name="psB", bufs=2,
                                                 space="PSUM"))
        self.psC = S_.enter_context(tc.tile_pool(name="psC", bufs=2,
                                                 space="PSUM"))
        self.awork = S_.enter_context(tc.tile_pool(name="awork", bufs=2))
        o_p = S_.enter_context(tc.tile_pool(name="oT", bufs=1))
        wp_p = S_.enter_context(tc.tile_pool(name="wpp", bufs=1))
        qkT_p = S_.enter_context(tc.tile_pool(name="qkT", bufs=1))
        v_p = S_.enter_context(tc.tile_pool(name="vtile", bufs=1))

        self.ident16 = const.tile([P, P], BF16, name="ident16")
        make_identity(nc, self.ident16[:])
        self.eps_t = const.tile([P, 1], F32, name="eps_t")
        nc.vector.memset(self.eps_t[:], EPS * S * S)
        if not self.bqk_zero:
            self.bqkv_sb = const.tile([P, 12], F32, name="bqkv_sb")
            nc.sync.dma_start(self.bqkv_sb[:], bqkv_d[:])
        if not self.b1_zero:
            self.b1_sb = const.tile([P, HS], F32, name="b1_sb")
            nc.sync.dma_start(self.b1_sb[:], b1_d[:])
        if not self.bv_zero:
            self.bv_bc = const.tile([P, C], F32, name="bv_bc")
            nc.sync.dma_start(self.bv_bc[:], bv_d[:].partition_broadcast(P))
        if not self.bproj_zero:
            self.bproj_bc = const.tile([P, C], F32, name="bproj_bc")
            nc.sync.dma_start(self.bproj_bc[:],
                              bproj_d[:].partition_broadcast(P))
        if not self.b2_zero:
            self.b2_bc = const.tile([P, C], F32, name="b2_bc")
            nc.sync.dma_start(self.b2_bc[:], b2_d[:].partition_broadcast(P))

        self.x_sb = xpool.tile([P, NT, C], F32, name="x_sb")
        xr = x_d[:].rearrange("(n p) c -> p n c", p=P)
        for t4 in range(4):
            nc.sync.dma_start(self.x_sb[:, t4 * 4:(t4 + 1) * 4, :],
                              xr[:, t4 * 4:(t4 + 1) * 4, :])

        wq_stack = ExitStack()
        wq_p = wq_stack.enter_context(tc.tile_pool(name="wqp", bufs=1))
        xnT_p = wq_stack.enter_context(tc.tile_pool(name="xnT1", bufs=2))
        self.wqkv_sb = wq_p.tile([P, KS, 3 * C], FP8, name="wqkv_sb")
        for c2 in range(3):
            nc.sync.dma_start(self.wqkv_sb[:, c2 * 2:c2 * 2 + 2, :],
                              wqkv_d[:, c2 * 2:c2 * 2 + 2, :])
        self.wproj_sb = wp_p.tile([P, KS, C], FP8, name="wproj_sb")
        nc.sync.dma_start(self.wproj_sb[:], wproj_d[:])

        outr = out_d[:].rearrange("(n p) c -> p n c", p=P)

        # ---------- seq 0: LN1 + V + (qk per head-pair || attention) ----------
        xnT0 = xnT_p.tile([P, KS, SEQ], FP8, tag="xnT", name="xnT0")
        qkT0 = qkT_p.tile([P, 12, SEQ], BF16, tag="qkT", name="qkT0")
        V0 = v_p.tile([P, NB, H, VP], FP8, tag="V", name="V0")
        oT0 = o_p.tile([P, KS, SEQ], FP8, tag="oT", name="oT0")
        for i in range(NB):
            self.ln_tile(self.x_sb[:, i, :], xnT0, i * P)
        nc.vector.memset(V0[:, :, :, HD], S)
        for t in range(NB):
            self.v_unit(t, xnT0, V0)

        # background: seq-1 LN1 pumped under seq-0 attention
        xnT1 = xnT_p.tile([P, KS, SEQ], FP8, tag="xnT", name="xnT1")
        bg = [(lambda i=i: self.ln_tile(self.x_sb[:, NB + i, :], xnT1,
                                        i * P))
              for i in range(NB)]
        nu = 0
        for p_ in range(6):
            for oct in (p_, 6 + p_):
                for nch in range(2):
                    self.qk_unit(oct, nch, xnT0, qkT0)
            for h in (2 * p_, 2 * p_ + 1):
                for qc in range(2):
                    self.attn_unit(h, qc, qkT0, V0, oT0)
                    nu += 1
                    if bg and nu % 3 == 0:
                        bg.pop(0)()
        while bg:
            bg.pop(0)()

        # ---------- valley 1: qkv(seq1), weights, proj+LN2(seq0), fc1 ----------
        V1 = v_p.tile([P, NB, H, VP], FP8, tag="V", name="V1")
        nc.vector.memset(V1[:, :, :, HD], S)
        for t in range(NB):
            self.v_unit(t, xnT1, V1)
        qkT1 = qkT_p.tile([P, 12, SEQ], BF16, tag="qkT", name="qkT1")
        for p_ in range(6):
            for oct in (p_, 6 + p_):
                for nch in range(2):
                    self.qk_unit(oct, nch, xnT1, qkT1)
        wq_stack.close()

        s4 = S_.enter_context(ExitStack())
        w_p = s4.enter_context(tc.tile_pool(name="wmlp", bufs=1))
        xnT2_p = s4.enter_context(tc.tile_pool(name="xnT2", bufs=1))
        h_p = s4.enter_context(tc.tile_pool(name="hT", bufs=2))
        self.w1_planes = []
        w1h_sb = w_p.tile([P, KS, HID], FP8, name="w1h_sb")
        for q in range(4):
            nc.sync.dma_start(w1h_sb[:, :, q * 768:(q + 1) * 768],
                              w1h_d[:, :, q * 768:(q + 1) * 768])
        self.w1_planes.append(w1h_sb)
        if self.w1x2:
            w1l_sb = w_p.tile([P, KS, HID], FP8, name="w1l_sb")
            for q in range(4):
                nc.sync.dma_start(w1l_sb[:, :, q * 768:(q + 1) * 768],
                                  w1l_d[:, :, q * 768:(q + 1) * 768])
            self.w1_planes.append(w1l_sb)
        self.w2_planes = []
        w2h_sb = w_p.tile([P, HS, C], FP8, name="w2h_sb")
        for q in range(4):
            nc.sync.dma_start(w2h_sb[:, q * 6:q * 6 + 6, :],
                              w2h_d[:, q * 6:q * 6 + 6, :])
        self.w2_planes.append(w2h_sb)
        if self.w2x2:
            w2l_sb = w_p.tile([P, HS, C], FP8, name="w2l_sb")
            for q in range(4):
                nc.sync.dma_start(w2l_sb[:, q * 6:q * 6 + 6, :],
                                  w2l_d[:, q * 6:q * 6 + 6, :])
            self.w2_planes.append(w2l_sb)

        xnT2_0 = xnT2_p.tile([P, KS, SEQ], FP8, tag="xnT2", name="xnT2_0")
        for t in range(NB):
            self.proj_tile(t, t, oT0)
            self.ln_tile(self.x_sb[:, t, :], xnT2_0, t * P)
        hT0 = h_p.tile([P, HS, 512], FP8, tag="hT", name="hT0")
        for hp in range(12):
            self.fc1_unit(hp, 0, xnT2_0, hT0)
        hT1 = h_p.tile([P, HS, 512], FP8, tag="hT", name="hT1")
        for hp in range(12):
            self.fc1_unit(hp, 512, xnT2_0, hT1)

        # ---------- seq 1 attention || fc2(seq0) + out DMA ----------
        oT1 = o_p.tile([P, KS, SEQ], FP8, tag="oT", name="oT1")
        bg = []
        for tt in range(4):
            bg.append(lambda tt=tt: self.fc2_unit(tt, tt, hT0))
        bg.append(lambda: nc.sync.dma_start(outr[:, 0:4, :],
                                            self.x_sb[:, 0:4, :]))
        for tt in range(4):
            bg.append(lambda tt=tt: self.fc2_unit(4 + tt, tt, hT1))
        bg.append(lambda: nc.sync.dma_start(outr[:, 4:8, :],
                                            self.x_sb[:, 4:8, :]))
        nu = 0
        for h in range(H):
            for qc in range(2):
                self.attn_unit(h, qc, qkT1, V1, oT1)
                nu += 1
                if bg and h >= 1 and nu % 2 == 0:
                    bg.pop(0)()
        while bg:
            bg.pop(0)()

        # ---------- tail: proj+LN2(seq1), fc1/fc2(tq2,3), out ----------
        xnT2_1 = xnT2_p.tile([P, KS, SEQ], FP8, tag="xnT2", name="xnT2_1")
        for t in range(NB):
            self.proj_tile(NB + t, t, oT1)
            self.ln_tile(self.x_sb[:, NB + t, :], xnT2_1, t * P)
        hT2 = h_p.tile([P, HS, 512], FP8, tag="hT", name="hT2")
        for hp in range(12):
            self.fc1_unit(hp, 0, xnT2_1, hT2)
        hT3 = h_p.tile([P, HS, 512], FP8, tag="hT", name="hT3")
        for hp in range(12):
            self.fc1_unit(hp, 512, xnT2_1, hT3)
        for tt in range(4):
            self.fc2_unit(8 + tt, tt, hT2)
        nc.sync.dma_start(outr[:, 8:12, :], self.x_sb[:, 8:12, :])
        for tt in range(4):
            self.fc2_unit(12 + tt, tt, hT3)
        nc.sync.dma_start(outr[:, 12:16, :], self.x_sb[:, 12:16, :])


def x_part(x_sb, tg, n0, nsz):
    return x_sb[:, tg, n0:n0 + nsz]


def _build(b1_zero=False, bv_zero=False, bproj_zero=False, b2_zero=False,
           bqk_zero=False, w1x2=True, w2x2=True):
    nc = bacc.Bacc(None, target_bir_lowering=False, debug=False)

    x_d = nc.dram_tensor("x", [T, C], F32, kind="ExternalInput")
    out_d = nc.dram_tensor("out", [T, C], F32, kind="ExternalOutput")
    wqkv_d = nc.dram_tensor("wqkv", [P, KS, 3 * C], FP8, kind="ExternalInput")
    bqkv_d = nc.dram_tensor("bqkv", [P, 12], F32, kind="ExternalInput")
    bv_d = nc.dram_tensor("bv", [C], F32, kind="ExternalInput")
    wproj_d = nc.dram_tensor("wproj", [P, KS, C], FP8, kind="ExternalInput")
    bproj_d = nc.dram_tensor("bproj", [C], F32, kind="ExternalInput")
    w1h_d = nc.dram_tensor("w1h", [P, KS, HID], FP8, kind="ExternalInput")
    w1l_d = nc.dram_tensor("w1l", [P, KS, HID], FP8, kind="ExternalInput")
    b1_d = nc.dram_tensor("b1", [P, HS], F32, kind="ExternalInput")
    w2h_d = nc.dram_tensor("w2h", [P, HS, C], FP8, kind="ExternalInput")
    w2l_d = nc.dram_tensor("w2l", [P, HS, C], FP8, kind="ExternalInput")
    b2_d = nc.dram_tensor("b2", [C], F32, kind="ExternalInput")
    with TileKernel(nc) as tk:
        tk.b1_zero = b1_zero
        tk.bqk_zero = bqk_zero
        tk.bv_zero = bv_zero
        tk.bproj_zero = bproj_zero
        tk.b2_zero = b2_zero
        tk.w1x2 = w1x2
        tk.w2x2 = w2x2
        tk.run(x_d, out_d, wqkv_d, bqkv_d, bv_d, wproj_d, bproj_d,
               w1h_d, w1l_d, b1_d, w2h_d, w2l_d, b2_d)

    nc.compile()
    return nc


def _hilo(w):
    hi = w.astype(NP8)
    lo = (w - hi.astype(np.float32)).astype(NP8)
    return hi, lo


def _prep_host(inputs):
    f = lambda a: np.asarray(a, dtype=np.float32)
    x = f(inputs["x"])
    ln1_g, ln1_b = f(inputs["ln1_g"]), f(inputs["ln1_b"])
    ln2_g, ln2_b = f(inputs["ln2_g"]), f(inputs["ln2_b"])
    qkv_w = f(inputs["qkv_w"])
    proj_w, proj_b = f(inputs["proj_w"]), f(inputs["proj_b"])
    fc1_w, fc1_b = f(inputs["fc1_w"]), f(inputs["fc1_b"])
    fc2_w, fc2_b = f(inputs["fc2_w"]), f(inputs["fc2_b"])

    wqkv = np.ascontiguousarray(
        (qkv_w * ln1_g[None, :] * S).T.reshape(KS, P, 3 * C).transpose(1, 0, 2)
    ).astype(NP8)
    bqkv_full = S * (qkv_w @ ln1_b)                # [2304], S-scaled
    bqkv = np.ascontiguousarray(bqkv_full[:2 * C].reshape(12, P).T)
    bv = np.ascontiguousarray(bqkv_full[2 * C:])
    wproj = np.ascontiguousarray(
        (proj_w * S).T.reshape(KS, P, C).transpose(1, 0, 2)).astype(NP8)
    w1 = np.ascontiguousarray(
        (fc1_w * ln2_g[None, :] * S).T.reshape(KS, P, HID).transpose(1, 0, 2))
    w1h, w1l = _hilo(w1)
    b1 = np.ascontiguousarray((fc1_b + fc1_w @ ln2_b).reshape(HS, P).T)
    w2 = np.ascontiguousarray(
        (fc2_w * S).T.reshape(HS, P, C).transpose(1, 0, 2))
    w2h, w2l = _hilo(w2)

    shared = {
        "wqkv": wqkv, "bqkv": bqkv, "bv": bv,
        "wproj": wproj, "bproj": S * proj_b,
        "w1h": w1h, "w1l": w1l, "b1": b1,
        "w2h": w2h, "w2l": w2l, "b2": S * fc2_b,
    }
    in_maps = []
    for c in range(8):
        m = dict(shared)
        m["x"] = np.ascontiguousarray(
            S * x[c * B_PER_CORE:(c + 1) * B_PER_CORE].reshape(T, C))
        in_maps.append(m)
    return in_maps


def kernel(**inputs):
    global _CACHED_NC
    b1_host = (np.asarray(inputs["fc1_b"], np.float32)
               + np.asarray(inputs["fc1_w"], np.float32)
               @ np.asarray(inputs["ln2_b"], np.float32))
    b1_zero = bool(np.all(b1_host == 0.0))
    bqkv_host = (np.asarray(inputs["qkv_w"], np.float32)
                 @ np.asarray(inputs["ln1_b"], np.float32))
    bv_zero = bool(np.all(bqkv_host[2 * C:] == 0.0))
    bqk_zero = bool(np.all(bqkv_host[:2 * C] == 0.0))
    bproj_zero = bool(np.all(np.asarray(inputs["proj_b"]) == 0.0))
    b2_zero = bool(np.all(np.asarray(inputs["fc2_b"]) == 0.0))
    key = (b1_zero, bv_zero, bproj_zero, b2_zero, bqk_zero)
    if _CACHED_NC is None or getattr(_CACHED_NC, "_spec", None) != key:
        _CACHED_NC = _build(b1_zero=b1_zero, bv_zero=bv_zero,
                            bproj_zero=bproj_zero, b2_zero=b2_zero,
                            bqk_zero=bqk_zero)
        _CACHED_NC._spec = key
    nc = _CACHED_NC
    in_maps = _prep_host(inputs)
    trace = os.environ.get("TRN_KERNEL_TRACE", "0") == "1"
    res = run_bass_kernel_spmd(nc, in_maps, core_ids=list(range(8)),
                               trace=trace)
    if trace and res.exec_time_ns is not None:
        print(f"HW exec time: {res.exec_time_ns} ns")
        print(f"mean exec time: {res.mean_exec_time_ns} ns")
        if res.instructions_and_trace is not None:
            print(f"trace: {res.instructions_and_trace[1]}")
    out = np.stack([
        res.results[c]["out"].reshape(B_PER_CORE, SEQ, C) for c in range(8)
    ]).reshape(16, SEQ, C)
    return (out / S).astype(np.float32)


# revision 13
# speedup vs baseline: 1.0150x; 1.0150x over previous
"""Trainium2 Bass kernel for a ViT-style transformer block (nn_Block_11132555231612).

Data-parallel over batch across 8 NeuronCores (2 sequences of 1024 tokens per
core). fp8e4m3 DoubleRow matmuls (0.5 cyc/row, 256-deep contraction) carry
QKV / AV / proj / fc1 / fc2; scores stay bf16. The residual stream is held at
32x scale (x scaled on host, weights scaled to match, output unscaled on
host) so fp8 weight scales fold away with no device fixups; the softmax
ones-column trick (column value = 32) cancels the V scale. fc1/fc2 weights
use hi+lo double-fp8 planes accumulated in PSUM for near-bf16 weight
precision.

The two sequences are pipelined as interleaved emission streams (engines
execute their queues in order, so overlap requires interleaving): seq-1
LayerNorm runs under seq-0 attention, and seq-0's fc2 (pure PE+DVE) runs
under seq-1's attention, whose critical resource is the scalar engine's
softmax exp. fc1+gelu run in the inter-attention valley where the scalar
engine is free. PSUM: A=[P,2,512]x2 (scores & fc1), B=[P,512]x2 (AV
accumulators), C=[P,512-slot]x2 (all other matmul outputs) = exactly 8 banks.
"""

import os
import sys

sys.path.insert(0, "/opt/trn_rl_repo")

import numpy as np
import ml_dtypes

import concourse.bass as bass
import concourse.mybir as mybir
import concourse.tile as tile
from concourse import bacc
from concourse.bass_utils import run_bass_kernel_spmd
from concourse.masks import make_identity
from contextlib import ExitStack

F32 = mybir.dt.float32
BF16 = mybir.dt.bfloat16
FP8 = mybir.dt.float8e4
NP8 = ml_dtypes.float8_e4m3
AF = mybir.ActivationFunctionType
DR = mybir.MatmulPerfMode.DoubleRow
ALU = mybir.AluOpType

P = 128
B_PER_CORE = 2
SEQ = 1024
T = B_PER_CORE * SEQ          # 2048 tokens per core
C = 768
H = 12
HD = 64
HID = 3072
KS = C // P                   # 6
HS = HID // P                 # 24
NT = T // P                   # 16 token tiles
NB = NT // B_PER_CORE         # 8 token tiles per sequence
EPS = 1e-5
SCALE = HD ** -0.5            # 0.125
S = 32.0                      # residual / weight scale
VP = 80                       # padded V row (65 used): 16B dual-fp8 ldweights

_CACHED_NC = None


class TileKernel:
    b1_zero = False
    bv_zero = False
    bproj_zero = False
    b2_zero = False
    bqk_zero = False
    w1x2 = True
    w2x2 = True

    def __init__(self, nc):
        self.nc = nc
        self.stack = ExitStack()
        self.tc = None

    def __enter__(self):
        self.tc = self.stack.enter_context(tile.TileContext(self.nc))
        return self

    def __exit__(self, *exc):
        return self.stack.__exit__(*exc)

    def ln_tile(self, xt, dst, dst_col):
        """LN of one token-major tile xt [P, C] -> feature-major columns
        dst[:, :, dst_col:dst_col+P]. Stats on DVE, apply on GPSIMD,
        transpose on PE (bf16; hw rejects fp8 transposes), psum->sbuf copy
        converts to dst dtype."""
        nc, work, psC = self.nc, self.work, self.psC
        st = work.tile([P, 3, 6], F32, tag="bnstats")
        xg = xt.rearrange("p (s d) -> p s d", s=3)
        for s in range(3):
            nc.vector.bn_stats(st[:, s, :], xg[:, s, :])
        mv = work.tile([P, 2], F32, tag="mv")
        nc.vector.bn_aggr(mv[:], st[:])
        sdv = work.tile([P, 1], F32, tag="sdv")
        nc.scalar.activation(sdv[:], mv[:, 1:2], AF.Sqrt, bias=self.eps_t[:])
        rstd = work.tile([P, 1], F32, tag="rstd")
        nc.vector.reciprocal(rstd[:], sdv[:])
        nmu = work.tile([P, 1], F32, tag="nmu")
        nc.vector.tensor_scalar_mul(nmu[:], mv[:, 0:1], -1.0)
        xn = work.tile([P, C], BF16, tag="xn")
        nc.gpsimd.tensor_scalar(xn[:], xt, nmu[:], rstd[:],
                                op0=ALU.add, op1=ALU.mult)
        for c in range(KS):
            pt = psC.tile([P, P], BF16, tag="C", name="pt")
            nc.tensor.transpose(pt[:], xn[:, c * P:(c + 1) * P],
                                self.ident16[:])
            nc.vector.tensor_copy(dst[:, c, dst_col:dst_col + P], pt[:])

    def v_unit(self, t, xnT_b, V_b):
        """V for one token tile (t local to the sequence) into V_b, via two
        C-ring psum chunks (heads 0-7, heads 8-11)."""
        nc, psC = self.nc, self.psC
        for (n0, nh) in ((0, 8), (512, 4)):
            psv = psC.tile([P, nh * HD], F32, tag="C", name="psv")
            for c2 in range(3):
                nc.tensor.matmul(
                    psv[:],
                    xnT_b[:, c2 * 2:c2 * 2 + 2, t * P:(t + 1) * P],
                    self.wqkv_sb[:, c2 * 2:c2 * 2 + 2,
                                 2 * C + n0:2 * C + n0 + nh * HD],
                    start=(c2 == 0), stop=(c2 == 2), perf_mode=DR)
            h0 = n0 // HD
            if self.bv_zero:
                nc.scalar.copy(
                    V_b[:, t, h0:h0 + nh, 0:HD],
                    psv[:].rearrange("p (h d) -> p h d", h=nh))
            else:
                nc.vector.tensor_add(
                    V_b[:, t, h0:h0 + nh, 0:HD],
                    psv[:].rearrange("p (h d) -> p h d", h=nh),
                    self.bv_bc[:, h0 * HD:(h0 + nh) * HD].rearrange(
                        "p (h d) -> p h d", h=nh))

    def qk_unit(self, oct, nch, xnT_b, qkT_b):
        """One q/k slice (oct) for one 512-token chunk of a sequence."""
        nc, psC = self.nc, self.psC
        ps = psC.tile([P, 512], F32, tag="C", name="ps")
        for c2 in range(3):
            nc.tensor.matmul(
                ps[:],
                self.wqkv_sb[:, c2 * 2:c2 * 2 + 2, oct * P:(oct + 1) * P],
                xnT_b[:, c2 * 2:c2 * 2 + 2, nch * 512:(nch + 1) * 512],
                start=(c2 == 0), stop=(c2 == 2), perf_mode=DR)
        if self.bqk_zero:
            nc.vector.tensor_copy(qkT_b[:, oct, nch * 512:(nch + 1) * 512],
                                  ps[:])
        else:
            nc.vector.tensor_scalar_add(
                qkT_b[:, oct, nch * 512:(nch + 1) * 512], ps[:],
                self.bqkv_sb[:, oct:oct + 1])

    def attn_unit(self, h, qc, qkT_b, V_b, oT_b):
        """Scores+softmax+AV+normalize for one (head, 512-query chunk)."""
        nc, psA, psB, awork = self.nc, self.psA, self.psB, self.awork
        po = (h % 2) * 64
        oq, ok = h // 2, 6 + h // 2
        qs = qc * 512
        pso = psB.tile([P, 512], F32, tag="B", name="pso")
        for kt2 in range(4):
            pss = psA.tile([P, 2, 512], F32, tag="A", name="pss")
            for j in range(2):
                ko = (2 * kt2 + j) * P
                nc.tensor.matmul(
                    pss[:, j, :],
                    qkT_b[po:po + HD, ok, ko:ko + P],
                    qkT_b[po:po + HD, oq, qs:qs + 512],
                    start=True, stop=True)
            pr = awork.tile([P, 2, 512], FP8, tag="probs")
            nc.scalar.activation(pr[:], pss[:], AF.Exp, scale=SCALE / (S * S))
            nc.tensor.matmul(
                pso[0:HD + 1, :],
                V_b[:, 2 * kt2:2 * kt2 + 2, h, 0:HD + 1],
                pr[:], start=(kt2 == 0), stop=(kt2 == 3), perf_mode=DR)
        rc = awork.tile([P, 512], BF16, tag="recip")
        with nc.allow_low_precision(reason="softmax denom reciprocal in bf16"):
            nc.vector.reciprocal(rc[HD:HD + 1, :], pso[HD:HD + 1, :])
        rc0 = awork.tile([1, 512], BF16, tag="rc0")
        nc.sync.dma_start(rc0[:], rc[HD:HD + 1, :])
        rbc = awork.tile([HD, 512], BF16, tag="rbc")
        nc.gpsimd.partition_broadcast(rbc[:], rc0[0:1, :], channels=HD)
        if h % 2 == 0:
            nc.vector.tensor_mul(oT_b[0:HD, h // 2, qs:qs + 512],
                                 pso[0:HD, :], rbc[:])
        else:
            osc = awork.tile([HD, 512], FP8, tag="osc")
            nc.vector.tensor_mul(osc[:], pso[0:HD, :], rbc[:])
            nc.sync.dma_start(oT_b[64:128, h // 2, qs:qs + 512], osc[:])

    def proj_tile(self, tg, lt, oT_b):
        """Attention projection + residual add for global tile tg (lt local
        within the sequence), then LN2 into xnT2 column."""
        nc, psC = self.nc, self.psC
        for (n0, nsz) in ((0, 512), (512, 256)):
            psp = psC.tile([P, nsz], F32, tag="C", name="psp")
            for c2 in range(3):
                nc.tensor.matmul(
                    psp[:],
                    oT_b[:, c2 * 2:c2 * 2 + 2, lt * P:(lt + 1) * P],
                    self.wproj_sb[:, c2 * 2:c2 * 2 + 2, n0:n0 + nsz],
                    start=(c2 == 0), stop=(c2 == 2), perf_mode=DR)
            nc.vector.tensor_add(x_part(self.x_sb, tg, n0, nsz),
                                 x_part(self.x_sb, tg, n0, nsz), psp[:])
            if not self.bproj_zero:
                nc.vector.tensor_add(
                    x_part(self.x_sb, tg, n0, nsz),
                    x_part(self.x_sb, tg, n0, nsz),
                    self.bproj_bc[:, n0:n0 + nsz])

    def fc1_unit(self, hp, tq_col, xnT2_b, hT_t):
        """fc1 + gelu for one hidden-feature pair over 512 tokens.
        tq_col: 0 or 512 (column offset within the sequence's xnT2)."""
        nc, psA = self.nc, self.psA
        n1 = 3 * len(self.w1_planes)
        ps1 = psA.tile([P, 2, 512], F32, tag="A", name="ps1")
        for j in range(2):
            i = 0
            for w1p in self.w1_planes:
                for c3 in range(3):
                    nc.tensor.matmul(
                        ps1[:, j, :],
                        w1p[:, c3 * 2:c3 * 2 + 2,
                            (hp * 2 + j) * P:(hp * 2 + j + 1) * P],
                        xnT2_b[:, c3 * 2:c3 * 2 + 2, tq_col:tq_col + 512],
                        start=(i == 0), stop=(i == n1 - 1), perf_mode=DR)
                    i += 1
        if self.b1_zero:
            nc.scalar.activation(
                hT_t[:, hp * 2:hp * 2 + 2, :].rearrange("p a b -> p (a b)"),
                ps1[:].rearrange("p a b -> p (a b)"), AF.Gelu, scale=1.0 / S)
        else:
            for j in range(2):
                nc.scalar.activation(
                    hT_t[:, hp * 2 + j, :], ps1[:, j, :], AF.Gelu,
                    bias=self.b1_sb[:, hp * 2 + j:hp * 2 + j + 1],
                    scale=1.0 / S)

    def fc2_unit(self, tg, tt, hT_t):
        """fc2 + residual add for global token tile tg (tt: 0..3 within the
        hT tile)."""
        nc, psC = self.nc, self.psC
        n2 = 12 * len(self.w2_planes)
        for (n0, nsz) in ((0, 512), (512, 256)):
            ps2 = psC.tile([P, nsz], F32, tag="C", name="ps2")
            i = 0
            for hp in range(12):
                for w2p in self.w2_planes:
                    nc.tensor.matmul(
                        ps2[:],
                        hT_t[:, hp * 2:hp * 2 + 2, tt * P:(tt + 1) * P],
                        w2p[:, hp * 2:hp * 2 + 2, n0:n0 + nsz],
                        start=(i == 0), stop=(i == n2 - 1), perf_mode=DR)
                    i += 1
            nc.vector.tensor_add(x_part(self.x_sb, tg, n0, nsz),
                                 x_part(self.x_sb, tg, n0, nsz), ps2[:])
            if not self.b2_zero:
                nc.vector.tensor_add(x_part(self.x_sb, tg, n0, nsz),
                                     x_part(self.x_sb, tg, n0, nsz),
                                     self.b2_bc[:, n0:n0 + nsz])

    def run(self, x_d, out_d, wqkv_d, bqkv_d, bv_d, wproj_d, bproj_d,
            w1h_d, w1l_d, b1_d, w2h_d, w2l_d, b2_d):
        nc, tc, S_ = self.nc, self.tc, self.stack
        const = S_.enter_context(tc.tile_pool(name="const", bufs=1))
        xpool = S_.enter_context(tc.tile_pool(name="xres", bufs=1))
        self.work = S_.enter_context(tc.tile_pool(name="work", bufs=2))
        self.psA = S_.enter_context(tc.tile_pool(name="psA", bufs=2,
                                                 space="PSUM"))
        self.psB = S_.enter_context(tc.tile_pool(name="psB", bufs=2,
                                                 space="PSUM"))
        self.psC = S_.enter_context(tc.tile_pool(name="psC", bufs=2,
                                                 space="PSUM"))
        self.awork = S_.enter_context(tc.tile_pool(name="awork", bufs=2))
        o_p = S_.enter_context(tc.tile_pool(name="oT", bufs=1))
        wp_p = S_.enter_context(tc.tile_pool(name="wpp", bufs=1))
        qkT_p = S_.enter_context(tc.tile_pool(name="qkT", bufs=1))
        v_p = S_.enter_context(tc.tile_pool(name="vtile", bufs=1))

        self.ident16 = const.tile([P, P], BF16, name="ident16")
        make_identity(nc, self.ident16[:])
        self.eps_t = const.tile([P, 1], F32, name="eps_t")
        nc.vector.memset(self.eps_t[:], EPS * S * S)
        if not self.bqk_zero:
            self.bqkv_sb = const.tile([P, 12], F32, name="bqkv_sb")
            nc.sync.dma_start(self.bqkv_sb[:], bqkv_d[:])
        if not self.b1_zero:
            self.b1_sb = const.tile([P, HS], F32, name="b1_sb")
            nc.sync.dma_start(self.b1_sb[:], b1_d[:])
        if not self.bv_zero:
            self.bv_bc = const.tile([P, C], F32, name="bv_bc")
            nc.sync.dma_start(self.bv_bc[:], bv_d[:].partition_broadcast(P))
        if not self.bproj_zero:
            self.bproj_bc = const.tile([P, C], F32, name="bproj_bc")
            nc.sync.dma_start(self.bproj_bc[:],
                              bproj_d[:].partition_broadcast(P))
        if not self.b2_zero:
            self.b2_bc = const.tile([P, C], F32, name="b2_bc")
            nc.sync.dma_start(self.b2_bc[:], b2_d[:].partition_broadcast(P))

        self.x_sb = xpool.tile([P, NT, C], F32, name="x_sb")
        xr = x_d[:].rearrange("(n p) c -> p n c", p=P)
        for t4 in range(4):
            nc.sync.dma_start(self.x_sb[:, t4 * 4:(t4 + 1) * 4, :],
                              xr[:, t4 * 4:(t4 + 1) * 4, :])

        wq_stack = ExitStack()
        wq_p = wq_stack.enter_context(tc.tile_pool(name="wqp", bufs=1))
        xnT_p = wq_stack.enter_context(tc.tile_pool(name="xnT1", bufs=2))
        self.wqkv_sb = wq_p.tile([P, KS, 3 * C], FP8, name="wqkv_sb")
        for c2 in range(3):
            nc.sync.dma_start(self.wqkv_sb[:, c2 * 2:c2 * 2 + 2, :],
                              wqkv_d[:, c2 * 2:c2 * 2 + 2, :])
        self.wproj_sb = wp_p.tile([P, KS, C], FP8, name="wproj_sb")
        nc.sync.dma_start(self.wproj_sb[:], wproj_d[:])

        outr = out_d[:].rearrange("(n p) c -> p n c", p=P)

        # ---------- seq 0: LN1 + V + (qk per head-pair || attention) ----------
        xnT0 = xnT_p.tile([P, KS, SEQ], FP8, tag="xnT", name="xnT0")
        qkT0 = qkT_p.tile([P, 12, SEQ], BF16, tag="qkT", name="qkT0")
        V0 = v_p.tile([P, NB, H, VP], FP8, tag="V", name="V0")
        oT0 = o_p.tile([P, KS, SEQ], FP8, tag="oT", name="oT0")
        for i in range(NB):
            self.ln_tile(self.x_sb[:, i, :], xnT0, i * P)
        nc.vector.memset(V0[:, :, :, HD], S)
        for t in range(NB):
            self.v_unit(t, xnT0, V0)

        # background: seq-1 LN1 pumped under seq-0 attention
        xnT1 = xnT_p.tile([P, KS, SEQ], FP8, tag="xnT", name="xnT1")
        bg = [(lambda i=i: self.ln_tile(self.x_sb[:, NB + i, :], xnT1,
                                        i * P))
              for i in range(NB)]
        nu = 0
        for p_ in range(6):
            for oct in (p_, 6 + p_):
                for nch in range(2):
                    self.qk_unit(oct, nch, xnT0, qkT0)
            for h in (2 * p_, 2 * p_ + 1):
                for qc in range(2):
                    self.attn_unit(h, qc, qkT0, V0, oT0)
                    nu += 1
                    if bg and nu % 3 == 0:
                        bg.pop(0)()
        while bg:
            bg.pop(0)()

        # ---------- valley 1: qkv(seq1), weights, proj+LN2(seq0), fc1 ----------
        V1 = v_p.tile([P, NB, H, VP], FP8, tag="V", name="V1")
        nc.vector.memset(V1[:, :, :, HD], S)
        for t in range(NB):
            self.v_unit(t, xnT1, V1)
        qkT1 = qkT_p.tile([P, 12, SEQ], BF16, tag="qkT", name="qkT1")
        for p_ in range(6):
            for oct in (p_, 6 + p_):
                for nch in range(2):
                    self.qk_unit(oct, nch, xnT1, qkT1)
        wq_stack.close()

        s4 = S_.enter_context(ExitStack())
        w_p = s4.enter_context(tc.tile_pool(name="wmlp", bufs=1))
        xnT2_p = s4.enter_context(tc.tile_pool(name="xnT2", bufs=1))
        h_p = s4.enter_context(tc.tile_pool(name="hT", bufs=2))
        self.w1_planes = []
        w1h_sb = w_p.tile([P, KS, HID], FP8, name="w1h_sb")
        for q in range(4):
            nc.sync.dma_start(w1h_sb[:, :, q * 768:(q + 1) * 768],
                              w1h_d[:, :, q * 768:(q + 1) * 768])
        self.w1_planes.append(w1h_sb)
        if self.w1x2:
            w1l_sb = w_p.tile([P, KS, HID], FP8, name="w1l_sb")
            for q in range(4):
                nc.sync.dma_start(w1l_sb[:, :, q * 768:(q + 1) * 768],
                                  w1l_d[:, :, q * 768:(q + 1) * 768])
            self.w1_planes.append(w1l_sb)
        self.w2_planes = []
        w2h_sb = w_p.tile([P, HS, C], FP8, name="w2h_sb")
        for q in range(4):
            nc.sync.dma_start(w2h_sb[:, q * 6:q * 6 + 6, :],
                              w2h_d[:, q * 6:q * 6 + 6, :])
        self.w2_planes.append(w2h_sb)
        if self.w2x2:
            w2l_sb = w_p.tile([P, HS, C], FP8, name="w2l_sb")
            for q in range(4):
                nc.sync.dma_start(w2l_sb[:, q * 6:q * 6 + 6, :],
                                  w2l_d[:, q * 6:q * 6 + 6, :])
            self.w2_planes.append(w2l_sb)

        xnT2_0 = xnT2_p.tile([P, KS, SEQ], FP8, tag="xnT2", name="xnT2_0")
        for t in range(NB):
            self.proj_tile(t, t, oT0)
            self.ln_tile(self.x_sb[:, t, :], xnT2_0, t * P)
        hT0 = h_p.tile([P, HS, 512], FP8, tag="hT", name="hT0")
        for hp in range(12):
            self.fc1_unit(hp, 0, xnT2_0, hT0)
        hT1 = h_p.tile([P, HS, 512], FP8, tag="hT", name="hT1")
        for hp in range(12):
            self.fc1_unit(hp, 512, xnT2_0, hT1)

        # ---------- seq 1 attention || fc2(seq0) + out DMA ----------
        oT1 = o_p.tile([P, KS, SEQ], FP8, tag="oT", name="oT1")
        bg = []
        for tt in range(4):
            bg.append(lambda tt=tt: self.fc2_unit(tt, tt, hT0))
        bg.append(lambda: nc.sync.dma_start(outr[:, 0:4, :],
                                            self.x_sb[:, 0:4, :]))
        for tt in range(4):
            bg.append(lambda tt=tt: self.fc2_unit(4 + tt, tt, hT1))
        bg.append(lambda: nc.sync.dma_start(outr[:, 4:8, :],
                                            self.x_sb[:, 4:8, :]))
        nu = 0
        for h in range(H):
            for qc in range(2):
                self.attn_unit(h, qc, qkT1, V1, oT1)
                nu += 1
                if bg and h >= 1 and nu % 2 == 0:
                    bg.pop(0)()
        while bg:
            bg.pop(0)()

        # ---------- tail: proj+LN2(seq1), fc1/fc2(tq2,3), out ----------
        xnT2_1 = xnT2_p.tile([P, KS, SEQ], FP8, tag="xnT2", name="xnT2_1")
        for t in range(NB):
            self.proj_tile(NB + t, t, oT1)
            self.ln_tile(self.x_sb[:, NB + t, :], xnT2_1, t * P)
        hT2 = h_p.tile([P, HS, 512], FP8, tag="hT", name="hT2")
        for hp in range(12):
            self.fc1_unit(hp, 0, xnT2_1, hT2)
        hT3 = h_p.tile([P, HS, 512], FP8, tag="hT", name="hT3")
        for hp in range(12):
            self.fc1_unit(hp, 512, xnT2_1, hT3)
        for tt in range(4):
            self.fc2_unit(8 + tt, tt, hT2)
        nc.sync.dma_start(outr[:, 8:12, :], self.x_sb[:, 8:12, :])
        for tt in range(4):
            self.fc2_unit(12 + tt, tt, hT3)
        nc.sync.dma_start(outr[:, 12:16, :], self.x_sb[:, 12:16, :])


def x_part(x_sb, tg, n0, nsz):
    return x_sb[:, tg, n0:n0 + nsz]


def _build(b1_zero=False, bv_zero=False, bproj_zero=False, b2_zero=False,
           bqk_zero=False, w1x2=True, w2x2=True):
    nc = bacc.Bacc(None, target_bir_lowering=False, debug=False)

    x_d = nc.dram_tensor("x", [T, C], F32, kind="ExternalInput")
    out_d = nc.dram_tensor("out", [T, C], F32, kind="ExternalOutput")
    wqkv_d = nc.dram_tensor("wqkv", [P, KS, 3 * C], FP8, kind="ExternalInput")
    bqkv_d = nc.dram_tensor("bqkv", [P, 12], F32, kind="ExternalInput")
    bv_d = nc.dram_tensor("bv", [C], F32, kind="ExternalInput")
    wproj_d = nc.dram_tensor("wproj", [P, KS, C], FP8, kind="ExternalInput")
    bproj_d = nc.dram_tensor("bproj", [C], F32, kind="ExternalInput")
    w1h_d = nc.dram_tensor("w1h", [P, KS, HID], FP8, kind="ExternalInput")
    w1l_d = nc.dram_tensor("w1l", [P, KS, HID], FP8, kind="ExternalInput")
    b1_d = nc.dram_tensor("b1", [P, HS], F32, kind="ExternalInput")
    w2h_d = nc.dram_tensor("w2h", [P, HS, C], FP8, kind="ExternalInput")
    w2l_d = nc.dram_tensor("w2l", [P, HS, C], FP8, kind="ExternalInput")
    b2_d = nc.dram_tensor("b2", [C], F32, kind="ExternalInput")
    with TileKernel(nc) as tk:
        tk.b1_zero = b1_zero
        tk.bqk_zero = bqk_zero
        tk.bv_zero = bv_zero
        tk.bproj_zero = bproj_zero
        tk.b2_zero = b2_zero
        tk.w1x2 = w1x2
        tk.w2x2 = w2x2
        tk.run(x_d, out_d, wqkv_d, bqkv_d, bv_d, wproj_d, bproj_d,
               w1h_d, w1l_d, b1_d, w2h_d, w2l_d, b2_d)

    nc.compile()
    return nc


def _hilo(w):
    hi = w.astype(NP8)
    lo = (w - hi.astype(np.float32)).astype(NP8)
    return hi, lo


def _prep_host(inputs):
    f = lambda a: np.asarray(a, dtype=np.float32)
    x = f(inputs["x"])
    ln1_g, ln1_b = f(inputs["ln1_g"]), f(inputs["ln1_b"])
    ln2_g, ln2_b = f(inputs["ln2_g"]), f(inputs["ln2_b"])
    qkv_w = f(inputs["qkv_w"])
    proj_w, proj_b = f(inputs["proj_w"]), f(inputs["proj_b"])
    fc1_w, fc1_b = f(inputs["fc1_w"]), f(inputs["fc1_b"])
    fc2_w, fc2_b = f(inputs["fc2_w"]), f(inputs["fc2_b"])

    wqkv = np.ascontiguousarray(
        (qkv_w * ln1_g[None, :] * S).T.reshape(KS, P, 3 * C).transpose(1, 0, 2)
    ).astype(NP8)
    bqkv_full = S * (qkv_w @ ln1_b)                # [2304], S-scaled
    bqkv = np.ascontiguousarray(bqkv_full[:2 * C].reshape(12, P).T)
    bv = np.ascontiguousarray(bqkv_full[2 * C:])
    wproj = np.ascontiguousarray(
        (proj_w * S).T.reshape(KS, P, C).transpose(1, 0, 2)).astype(NP8)
    w1 = np.ascontiguousarray(
        (fc1_w * ln2_g[None, :] * S).T.reshape(KS, P, HID).transpose(1, 0, 2))
    w1h, w1l = _hilo(w1)
    b1 = np.ascontiguousarray((fc1_b + fc1_w @ ln2_b).reshape(HS, P).T)
    w2 = np.ascontiguousarray(
        (fc2_w * S).T.reshape(HS, P, C).transpose(1, 0, 2))
    w2h, w2l = _hilo(w2)

    shared = {
        "wqkv": wqkv, "bqkv": bqkv, "bv": bv,
        "wproj": wproj, "bproj": S * proj_b,
        "w1h": w1h, "w1l": w1l, "b1": b1,
        "w2h": w2h, "w2l": w2l, "b2": S * fc2_b,
    }
    in_maps = []
    for c in range(8):
        m = dict(shared)
        m["x"] = np.ascontiguousarray(
            S * x[c * B_PER_CORE:(c + 1) * B_PER_CORE].reshape(T, C))
        in_maps.append(m)
    return in_maps


def kernel(**inputs):
    global _CACHED_NC
    b1_host = (np.asarray(inputs["fc1_b"], np.float32)
               + np.asarray(inputs["fc1_w"], np.float32)
               @ np.asarray(inputs["ln2_b"], np.float32))
    b1_zero = bool(np.all(b1_host == 0.0))
    bqkv_host = (np.asarray(inputs["qkv_w"], np.float32)
                 @ np.asarray(inputs["ln1_b"], np.float32))
    bv_zero = bool(np.all(bqkv_host[2 * C:] == 0.0))
    bqk_zero = bool(np.all(bqkv_host[:2 * C] == 0.0))
    bproj_zero = bool(np.all(np.asarray(inputs["proj_b"]) == 0.0))
    b2_zero = bool(np.all(np.asarray(inputs["fc2_b"]) == 0.0))
    key = (b1_zero, bv_zero, bproj_zero, b2_zero, bqk_zero)
    if _CACHED_NC is None or getattr(_CACHED_NC, "_spec", None) != key:
        _CACHED_NC = _build(b1_zero=b1_zero, bv_zero=bv_zero,
                            bproj_zero=bproj_zero, b2_zero=b2_zero,
                            bqk_zero=bqk_zero)
        _CACHED_NC._spec = key
    nc = _CACHED_NC
    in_maps = _prep_host(inputs)
    trace = os.environ.get("TRN_KERNEL_TRACE", "0") == "1"
    res = run_bass_kernel_spmd(nc, in_maps, core_ids=list(range(8)),
                               trace=trace)
    if trace and res.exec_time_ns is not None:
        print(f"HW exec time: {res.exec_time_ns} ns")
        print(f"mean exec time: {res.mean_exec_time_ns} ns")
        if res.instructions_and_trace is not None:
            print(f"trace: {res.instructions_and_trace[1]}")
    out = np.stack([
        res.results[c]["out"].reshape(B_PER_CORE, SEQ, C) for c in range(8)
    ]).reshape(16, SEQ, C)
    return (out / S).astype(np.float32)


# revision 14
# speedup vs baseline: 1.0183x; 1.0032x over previous
"""Trainium2 Bass kernel for a ViT-style transformer block (nn_Block_11132555231612).

Data-parallel over batch across 8 NeuronCores (2 sequences of 1024 tokens per
core). fp8e4m3 DoubleRow matmuls (0.5 cyc/row, 256-deep contraction) carry
QKV / AV / proj / fc1 / fc2; scores stay bf16. The residual stream is held at
32x scale (x scaled on host, weights scaled to match, output unscaled on
host) so fp8 weight scales fold away with no device fixups; the softmax
ones-column trick (column value = 32) cancels the V scale. fc1/fc2 weights
use hi+lo double-fp8 planes accumulated in PSUM for near-bf16 weight
precision.

The two sequences are pipelined as interleaved emission streams (engines
execute their queues in order, so overlap requires interleaving): seq-1
LayerNorm runs under seq-0 attention, and seq-0's fc2 (pure PE+DVE) runs
under seq-1's attention, whose critical resource is the scalar engine's
softmax exp. fc1+gelu run in the inter-attention valley where the scalar
engine is free. PSUM: A=[P,2,512]x2 (scores & fc1), B=[P,512]x2 (AV
accumulators), C=[P,512-slot]x2 (all other matmul outputs) = exactly 8 banks.
"""

import os
import sys

sys.path.insert(0, "/opt/trn_rl_repo")

import numpy as np
import ml_dtypes

import concourse.bass as bass
import concourse.mybir as mybir
import concourse.tile as tile
from concourse import bacc
from concourse.bass_utils import run_bass_kernel_spmd
from concourse.masks import make_identity
from contextlib import ExitStack

F32 = mybir.dt.float32
BF16 = mybir.dt.bfloat16
FP8 = mybir.dt.float8e4
NP8 = ml_dtypes.float8_e4m3
AF = mybir.ActivationFunctionType
DR = mybir.MatmulPerfMode.DoubleRow
ALU = mybir.AluOpType

P = 128
B_PER_CORE = 2
SEQ = 1024
T = B_PER_CORE * SEQ          # 2048 tokens per core
C = 768
H = 12
HD = 64
HID = 3072
KS = C // P                   # 6
HS = HID // P                 # 24
NT = T // P                   # 16 token tiles
NB = NT // B_PER_CORE         # 8 token tiles per sequence
EPS = 1e-5
SCALE = HD ** -0.5            # 0.125
S = 32.0                      # residual / weight scale
VP = 80                       # padded V row (65 used): 16B dual-fp8 ldweights

_CACHED_NC = None


class TileKernel:
    b1_zero = False
    bv_zero = False
    bproj_zero = False
    b2_zero = False
    bqk_zero = False
    w1x2 = True
    w2x2 = True

    def __init__(self, nc):
        self.nc = nc
        self.stack = ExitStack()
        self.tc = None

    def __enter__(self):
        self.tc = self.stack.enter_context(tile.TileContext(self.nc))
        return self

    def __exit__(self, *exc):
        return self.stack.__exit__(*exc)

    def ln_tile(self, xt, dst, dst_col):
        """LN of one token-major tile xt [P, C] -> feature-major columns
        dst[:, :, dst_col:dst_col+P]. Stats on DVE, apply on GPSIMD,
        transpose on PE (bf16; hw rejects fp8 transposes), psum->sbuf copy
        converts to dst dtype."""
        nc, work, psC = self.nc, self.work, self.psC
        st = work.tile([P, 3, 6], F32, tag="bnstats")
        xg = xt.rearrange("p (s d) -> p s d", s=3)
        for s in range(3):
            nc.vector.bn_stats(st[:, s, :], xg[:, s, :])
        mv = work.tile([P, 2], F32, tag="mv")
        nc.vector.bn_aggr(mv[:], st[:])
        sdv = work.tile([P, 1], F32, tag="sdv")
        nc.scalar.activation(sdv[:], mv[:, 1:2], AF.Sqrt, bias=self.eps_t[:])
        rstd = work.tile([P, 1], F32, tag="rstd")
        nc.vector.reciprocal(rstd[:], sdv[:])
        nmu = work.tile([P, 1], F32, tag="nmu")
        nc.vector.tensor_scalar_mul(nmu[:], mv[:, 0:1], -1.0)
        xn = work.tile([P, C], BF16, tag="xn")
        nc.gpsimd.tensor_scalar(xn[:], xt, nmu[:], rstd[:],
                                op0=ALU.add, op1=ALU.mult)
        for c in range(KS):
            pt = psC.tile([P, P], BF16, tag="C", name="pt")
            nc.tensor.transpose(pt[:], xn[:, c * P:(c + 1) * P],
                                self.ident16[:])
            nc.vector.tensor_copy(dst[:, c, dst_col:dst_col + P], pt[:])

    def v_unit(self, t, xnT_b, V_b):
        """V for one token tile (t local to the sequence) into V_b, via two
        C-ring psum chunks (heads 0-7, heads 8-11)."""
        nc, psC = self.nc, self.psC
        for (n0, nh) in ((0, 8), (512, 4)):
            psv = psC.tile([P, nh * HD], F32, tag="C", name="psv")
            for c2 in range(3):
                nc.tensor.matmul(
                    psv[:],
                    xnT_b[:, c2 * 2:c2 * 2 + 2, t * P:(t + 1) * P],
                    self.wqkv_sb[:, c2 * 2:c2 * 2 + 2,
                                 2 * C + n0:2 * C + n0 + nh * HD],
                    start=(c2 == 0), stop=(c2 == 2), perf_mode=DR)
            h0 = n0 // HD
            if self.bv_zero:
                nc.scalar.copy(
                    V_b[:, t, h0:h0 + nh, 0:HD],
                    psv[:].rearrange("p (h d) -> p h d", h=nh))
            else:
                nc.vector.tensor_add(
                    V_b[:, t, h0:h0 + nh, 0:HD],
                    psv[:].rearrange("p (h d) -> p h d", h=nh),
                    self.bv_bc[:, h0 * HD:(h0 + nh) * HD].rearrange(
                        "p (h d) -> p h d", h=nh))

    def qk_unit(self, oct, nch, xnT_b, qkT_b):
        """One q/k slice (oct) for one 512-token chunk of a sequence."""
        nc, psC = self.nc, self.psC
        ps = psC.tile([P, 512], F32, tag="C", name="ps")
        for c2 in range(3):
            nc.tensor.matmul(
                ps[:],
                self.wqkv_sb[:, c2 * 2:c2 * 2 + 2, oct * P:(oct + 1) * P],
                xnT_b[:, c2 * 2:c2 * 2 + 2, nch * 512:(nch + 1) * 512],
                start=(c2 == 0), stop=(c2 == 2), perf_mode=DR)
        if self.bqk_zero:
            nc.vector.tensor_copy(qkT_b[:, oct, nch * 512:(nch + 1) * 512],
                                  ps[:])
        else:
            nc.vector.tensor_scalar_add(
                qkT_b[:, oct, nch * 512:(nch + 1) * 512], ps[:],
                self.bqkv_sb[:, oct:oct + 1])

    def attn_unit(self, h, qc, qkT_b, V_b, oT_b):
        """Scores+softmax+AV+normalize for one (head, 512-query chunk)."""
        nc, psA, psB, awork = self.nc, self.psA, self.psB, self.awork
        po = (h % 2) * 64
        oq, ok = h // 2, 6 + h // 2
        qs = qc * 512
        pso = psB.tile([P, 512], F32, tag="B", name="pso")
        for kt2 in range(4):
            pss = psA.tile([P, 2, 512], F32, tag="A", name="pss")
            for j in range(2):
                ko = (2 * kt2 + j) * P
                nc.tensor.matmul(
                    pss[:, j, :],
                    qkT_b[po:po + HD, ok, ko:ko + P],
                    qkT_b[po:po + HD, oq, qs:qs + 512],
                    start=True, stop=True)
            pr = awork.tile([P, 2, 512], FP8, tag="probs", bufs=3)
            nc.scalar.activation(pr[:], pss[:], AF.Exp, scale=SCALE / (S * S))
            nc.tensor.matmul(
                pso[0:HD + 1, :],
                V_b[:, 2 * kt2:2 * kt2 + 2, h, 0:HD + 1],
                pr[:], start=(kt2 == 0), stop=(kt2 == 3), perf_mode=DR)
        rc = awork.tile([P, 512], BF16, tag="recip")
        with nc.allow_low_precision(reason="softmax denom reciprocal in bf16"):
            nc.vector.reciprocal(rc[HD:HD + 1, :], pso[HD:HD + 1, :])
        rc0 = awork.tile([1, 512], BF16, tag="rc0")
        nc.sync.dma_start(rc0[:], rc[HD:HD + 1, :])
        rbc = awork.tile([HD, 512], BF16, tag="rbc")
        nc.gpsimd.partition_broadcast(rbc[:], rc0[0:1, :], channels=HD)
        if h % 2 == 0:
            nc.vector.tensor_mul(oT_b[0:HD, h // 2, qs:qs + 512],
                                 pso[0:HD, :], rbc[:])
        else:
            osc = awork.tile([HD, 512], FP8, tag="osc")
            nc.vector.tensor_mul(osc[:], pso[0:HD, :], rbc[:])
            nc.sync.dma_start(oT_b[64:128, h // 2, qs:qs + 512], osc[:])

    def proj_tile(self, tg, lt, oT_b):
        """Attention projection + residual add for global tile tg (lt local
        within the sequence), then LN2 into xnT2 column."""
        nc, psC = self.nc, self.psC
        for (n0, nsz) in ((0, 512), (512, 256)):
            psp = psC.tile([P, nsz], F32, tag="C", name="psp")
            for c2 in range(3):
                nc.tensor.matmul(
                    psp[:],
                    oT_b[:, c2 * 2:c2 * 2 + 2, lt * P:(lt + 1) * P],
                    self.wproj_sb[:, c2 * 2:c2 * 2 + 2, n0:n0 + nsz],
                    start=(c2 == 0), stop=(c2 == 2), perf_mode=DR)
            nc.vector.tensor_add(x_part(self.x_sb, tg, n0, nsz),
                                 x_part(self.x_sb, tg, n0, nsz), psp[:])
            if not self.bproj_zero:
                nc.vector.tensor_add(
                    x_part(self.x_sb, tg, n0, nsz),
                    x_part(self.x_sb, tg, n0, nsz),
                    self.bproj_bc[:, n0:n0 + nsz])

    def fc1_unit(self, hp, tq_col, xnT2_b, hT_t):
        """fc1 + gelu for one hidden-feature pair over 512 tokens.
        tq_col: 0 or 512 (column offset within the sequence's xnT2)."""
        nc, psA = self.nc, self.psA
        n1 = 3 * len(self.w1_planes)
        ps1 = psA.tile([P, 2, 512], F32, tag="A", name="ps1")
        for j in range(2):
            i = 0
            for w1p in self.w1_planes:
                for c3 in range(3):
                    nc.tensor.matmul(
                        ps1[:, j, :],
                        w1p[:, c3 * 2:c3 * 2 + 2,
                            (hp * 2 + j) * P:(hp * 2 + j + 1) * P],
                        xnT2_b[:, c3 * 2:c3 * 2 + 2, tq_col:tq_col + 512],
                        start=(i == 0), stop=(i == n1 - 1), perf_mode=DR)
                    i += 1
        if self.b1_zero:
            nc.scalar.activation(
                hT_t[:, hp * 2:hp * 2 + 2, :].rearrange("p a b -> p (a b)"),
                ps1[:].rearrange("p a b -> p (a b)"), AF.Gelu, scale=1.0 / S)
        else:
            for j in range(2):
                nc.scalar.activation(
                    hT_t[:, hp * 2 + j, :], ps1[:, j, :], AF.Gelu,
                    bias=self.b1_sb[:, hp * 2 + j:hp * 2 + j + 1],
                    scale=1.0 / S)

    def fc2_unit(self, tg, tt, hT_t):
        """fc2 + residual add for global token tile tg (tt: 0..3 within the
        hT tile)."""
        nc, psC = self.nc, self.psC
        n2 = 12 * len(self.w2_planes)
        for (n0, nsz) in ((0, 512), (512, 256)):
            ps2 = psC.tile([P, nsz], F32, tag="C", name="ps2")
            i = 0
            for hp in range(12):
                for w2p in self.w2_planes:
                    nc.tensor.matmul(
                        ps2[:],
                        hT_t[:, hp * 2:hp * 2 + 2, tt * P:(tt + 1) * P],
                        w2p[:, hp * 2:hp * 2 + 2, n0:n0 + nsz],
                        start=(i == 0), stop=(i == n2 - 1), perf_mode=DR)
                    i += 1
            nc.vector.tensor_add(x_part(self.x_sb, tg, n0, nsz),
                                 x_part(self.x_sb, tg, n0, nsz), ps2[:])
            if not self.b2_zero:
                nc.vector.tensor_add(x_part(self.x_sb, tg, n0, nsz),
                                     x_part(self.x_sb, tg, n0, nsz),
                                     self.b2_bc[:, n0:n0 + nsz])

    def run(self, x_d, out_d, wqkv_d, bqkv_d, bv_d, wproj_d, bproj_d,
            w1h_d, w1l_d, b1_d, w2h_d, w2l_d, b2_d):
        nc, tc, S_ = self.nc, self.tc, self.stack
        const = S_.enter_context(tc.tile_pool(name="const", bufs=1))
        xpool = S_.enter_context(tc.tile_pool(name="xres", bufs=1))
        self.work = S_.enter_context(tc.tile_pool(name="work", bufs=2))
        self.psA = S_.enter_context(tc.tile_pool(name="psA", bufs=2,
                                                 space="PSUM"))
        self.psB = S_.enter_context(tc.tile_pool(name="psB", bufs=2,
                                                 space="PSUM"))
        self.psC = S_.enter_context(tc.tile_pool(name="psC", bufs=2,
                                                 space="PSUM"))
        self.awork = S_.enter_context(tc.tile_pool(name="awork", bufs=2))
        o_p = S_.enter_context(tc.tile_pool(name="oT", bufs=1))
        wp_p = S_.enter_context(tc.tile_pool(name="wpp", bufs=1))
        qkT_p = S_.enter_context(tc.tile_pool(name="qkT", bufs=1))
        v_p = S_.enter_context(tc.tile_pool(name="vtile", bufs=1))

        self.ident16 = const.tile([P, P], BF16, name="ident16")
        make_identity(nc, self.ident16[:])
        self.eps_t = const.tile([P, 1], F32, name="eps_t")
        nc.vector.memset(self.eps_t[:], EPS * S * S)
        if not self.bqk_zero:
            self.bqkv_sb = const.tile([P, 12], F32, name="bqkv_sb")
            nc.sync.dma_start(self.bqkv_sb[:], bqkv_d[:])
        if not self.b1_zero:
            self.b1_sb = const.tile([P, HS], F32, name="b1_sb")
            nc.sync.dma_start(self.b1_sb[:], b1_d[:])
        if not self.bv_zero:
            self.bv_bc = const.tile([P, C], F32, name="bv_bc")
            nc.sync.dma_start(self.bv_bc[:], bv_d[:].partition_broadcast(P))
        if not self.bproj_zero:
            self.bproj_bc = const.tile([P, C], F32, name="bproj_bc")
            nc.sync.dma_start(self.bproj_bc[:],
                              bproj_d[:].partition_broadcast(P))
        if not self.b2_zero:
            self.b2_bc = const.tile([P, C], F32, name="b2_bc")
            nc.sync.dma_start(self.b2_bc[:], b2_d[:].partition_broadcast(P))

        self.x_sb = xpool.tile([P, NT, C], F32, name="x_sb")
        xr = x_d[:].rearrange("(n p) c -> p n c", p=P)
        for t4 in range(4):
            nc.sync.dma_start(self.x_sb[:, t4 * 4:(t4 + 1) * 4, :],
                              xr[:, t4 * 4:(t4 + 1) * 4, :])

        wq_stack = ExitStack()
        wq_p = wq_stack.enter_context(tc.tile_pool(name="wqp", bufs=1))
        xnT_p = wq_stack.enter_context(tc.tile_pool(name="xnT1", bufs=2))
        self.wqkv_sb = wq_p.tile([P, KS, 3 * C], FP8, name="wqkv_sb")
        for c2 in range(3):
            nc.sync.dma_start(self.wqkv_sb[:, c2 * 2:c2 * 2 + 2, :],
                              wqkv_d[:, c2 * 2:c2 * 2 + 2, :])
        self.wproj_sb = wp_p.tile([P, KS, C], FP8, name="wproj_sb")
        nc.sync.dma_start(self.wproj_sb[:], wproj_d[:])

        outr = out_d[:].rearrange("(n p) c -> p n c", p=P)

        # ---------- seq 0: LN1 + V + (qk per head-pair || attention) ----------
        xnT0 = xnT_p.tile([P, KS, SEQ], FP8, tag="xnT", name="xnT0")
        qkT0 = qkT_p.tile([P, 12, SEQ], BF16, tag="qkT", name="qkT0")
        V0 = v_p.tile([P, NB, H, VP], FP8, tag="V", name="V0")
        oT0 = o_p.tile([P, KS, SEQ], FP8, tag="oT", name="oT0")
        for i in range(NB):
            self.ln_tile(self.x_sb[:, i, :], xnT0, i * P)
        nc.vector.memset(V0[:, :, :, HD], S)
        for t in range(NB):
            self.v_unit(t, xnT0, V0)

        # background: seq-1 LN1 pumped under seq-0 attention
        xnT1 = xnT_p.tile([P, KS, SEQ], FP8, tag="xnT", name="xnT1")
        bg = [(lambda i=i: self.ln_tile(self.x_sb[:, NB + i, :], xnT1,
                                        i * P))
              for i in range(NB)]
        nu = 0
        for p_ in range(6):
            for oct in (p_, 6 + p_):
                for nch in range(2):
                    self.qk_unit(oct, nch, xnT0, qkT0)
            for h in (2 * p_, 2 * p_ + 1):
                for qc in range(2):
                    self.attn_unit(h, qc, qkT0, V0, oT0)
                    nu += 1
                    if bg and nu % 3 == 0:
                        bg.pop(0)()
        while bg:
            bg.pop(0)()

        # ---------- valley 1: qkv(seq1), weights, proj+LN2(seq0), fc1 ----------
        V1 = v_p.tile([P, NB, H, VP], FP8, tag="V", name="V1")
        nc.vector.memset(V1[:, :, :, HD], S)
        for t in range(NB):
            self.v_unit(t, xnT1, V1)
        qkT1 = qkT_p.tile([P, 12, SEQ], BF16, tag="qkT", name="qkT1")
        for p_ in range(6):
            for oct in (p_, 6 + p_):
                for nch in range(2):
                    self.qk_unit(oct, nch, xnT1, qkT1)
        wq_stack.close()

        s4 = S_.enter_context(ExitStack())
        w_p = s4.enter_context(tc.tile_pool(name="wmlp", bufs=1))
        xnT2_p = s4.enter_context(tc.tile_pool(name="xnT2", bufs=1))
        h_p = s4.enter_context(tc.tile_pool(name="hT", bufs=2))
        self.w1_planes = []
        w1h_sb = w_p.tile([P, KS, HID], FP8, name="w1h_sb")
        for q in range(4):
            nc.sync.dma_start(w1h_sb[:, :, q * 768:(q + 1) * 768],
                              w1h_d[:, :, q * 768:(q + 1) * 768])
        self.w1_planes.append(w1h_sb)
        if self.w1x2:
            w1l_sb = w_p.tile([P, KS, HID], FP8, name="w1l_sb")
            for q in range(4):
                nc.sync.dma_start(w1l_sb[:, :, q * 768:(q + 1) * 768],
                                  w1l_d[:, :, q * 768:(q + 1) * 768])
            self.w1_planes.append(w1l_sb)
        self.w2_planes = []
        w2h_sb = w_p.tile([P, HS, C], FP8, name="w2h_sb")
        for q in range(4):
            nc.sync.dma_start(w2h_sb[:, q * 6:q * 6 + 6, :],
                              w2h_d[:, q * 6:q * 6 + 6, :])
        self.w2_planes.append(w2h_sb)
        if self.w2x2:
            w2l_sb = w_p.tile([P, HS, C], FP8, name="w2l_sb")
            for q in range(4):
                nc.sync.dma_start(w2l_sb[:, q * 6:q * 6 + 6, :],
                                  w2l_d[:, q * 6:q * 6 + 6, :])
            self.w2_planes.append(w2l_sb)

        xnT2_0 = xnT2_p.tile([P, KS, SEQ], FP8, tag="xnT2", name="xnT2_0")
        for t in range(NB):
            self.proj_tile(t, t, oT0)
            self.ln_tile(self.x_sb[:, t, :], xnT2_0, t * P)
        hT0 = h_p.tile([P, HS, 512], FP8, tag="hT", name="hT0")
        for hp in range(12):
            self.fc1_unit(hp, 0, xnT2_0, hT0)
        hT1 = h_p.tile([P, HS, 512], FP8, tag="hT", name="hT1")
        for hp in range(12):
            self.fc1_unit(hp, 512, xnT2_0, hT1)

        # ---------- seq 1 attention || fc2(seq0) + out DMA ----------
        oT1 = o_p.tile([P, KS, SEQ], FP8, tag="oT", name="oT1")
        bg = []
        for tt in range(4):
            bg.append(lambda tt=tt: self.fc2_unit(tt, tt, hT0))
        bg.append(lambda: nc.sync.dma_start(outr[:, 0:4, :],
                                            self.x_sb[:, 0:4, :]))
        for tt in range(4):
            bg.append(lambda tt=tt: self.fc2_unit(4 + tt, tt, hT1))
        bg.append(lambda: nc.sync.dma_start(outr[:, 4:8, :],
                                            self.x_sb[:, 4:8, :]))
        nu = 0
        for h in range(H):
            for qc in range(2):
                self.attn_unit(h, qc, qkT1, V1, oT1)
                nu += 1
                if bg and h >= 1 and nu % 2 == 0:
                    bg.pop(0)()
        while bg:
            bg.pop(0)()

        # ---------- tail: proj+LN2(seq1), fc1/fc2(tq2,3), out ----------
        xnT2_1 = xnT2_p.tile([P, KS, SEQ], FP8, tag="xnT2", name="xnT2_1")
        for t in range(NB):
            self.proj_tile(NB + t, t, oT1)
            self.ln_tile(self.x_sb[:, NB + t, :], xnT2_1, t * P)
        hT2 = h_p.tile([P, HS, 512], FP8, tag="hT", name="hT2")
        for hp in range(12):
            self.fc1_unit(hp, 0, xnT2_1, hT2)
        hT3 = h_p.tile([P, HS, 512], FP8, tag="hT", name="hT3")
        for hp in range(12):
            self.fc1_unit(hp, 512, xnT2_1, hT3)
        for tt in range(4):
            self.fc2_unit(8 + tt, tt, hT2)
        nc.sync.dma_start(outr[:, 8:12, :], self.x_sb[:, 8:12, :])
        for tt in range(4):
            self.fc2_unit(12 + tt, tt, hT3)
        nc.sync.dma_start(outr[:, 12:16, :], self.x_sb[:, 12:16, :])


def x_part(x_sb, tg, n0, nsz):
    return x_sb[:, tg, n0:n0 + nsz]


def _build(b1_zero=False, bv_zero=False, bproj_zero=False, b2_zero=False,
           bqk_zero=False, w1x2=True, w2x2=True):
    nc = bacc.Bacc(None, target_bir_lowering=False, debug=False)

    x_d = nc.dram_tensor("x", [T, C], F32, kind="ExternalInput")
    out_d = nc.dram_tensor("out", [T, C], F32, kind="ExternalOutput")
    wqkv_d = nc.dram_tensor("wqkv", [P, KS, 3 * C], FP8, kind="ExternalInput")
    bqkv_d = nc.dram_tensor("bqkv", [P, 12], F32, kind="ExternalInput")
    bv_d = nc.dram_tensor("bv", [C], F32, kind="ExternalInput")
    wproj_d = nc.dram_tensor("wproj", [P, KS, C], FP8, kind="ExternalInput")
    bproj_d = nc.dram_tensor("bproj", [C], F32, kind="ExternalInput")
    w1h_d = nc.dram_tensor("w1h", [P, KS, HID], FP8, kind="ExternalInput")
    w1l_d = nc.dram_tensor("w1l", [P, KS, HID], FP8, kind="ExternalInput")
    b1_d = nc.dram_tensor("b1", [P, HS], F32, kind="ExternalInput")
    w2h_d = nc.dram_tensor("w2h", [P, HS, C], FP8, kind="ExternalInput")
    w2l_d = nc.dram_tensor("w2l", [P, HS, C], FP8, kind="ExternalInput")
    b2_d = nc.dram_tensor("b2", [C], F32, kind="ExternalInput")
    with TileKernel(nc) as tk:
        tk.b1_zero = b1_zero
        tk.bqk_zero = bqk_zero
        tk.bv_zero = bv_zero
        tk.bproj_zero = bproj_zero
        tk.b2_zero = b2_zero
        tk.w1x2 = w1x2
        tk.w2x2 = w2x2
        tk.run(x_d, out_d, wqkv_d, bqkv_d, bv_d, wproj_d, bproj_d,
               w1h_d, w1l_d, b1_d, w2h_d, w2l_d, b2_d)

    nc.compile()
    return nc


def _hilo(w):
    hi = w.astype(NP8)
    lo = (w - hi.astype(np.float32)).astype(NP8)
    return hi, lo


def _prep_host(inputs):
    f = lambda a: np.asarray(a, dtype=np.float32)
    x = f(inputs["x"])
    ln1_g, ln1_b = f(inputs["ln1_g"]), f(inputs["ln1_b"])
    ln2_g, ln2_b = f(inputs["ln2_g"]), f(inputs["ln2_b"])
    qkv_w = f(inputs["qkv_w"])
    proj_w, proj_b = f(inputs["proj_w"]), f(inputs["proj_b"])
    fc1_w, fc1_b = f(inputs["fc1_w"]), f(inputs["fc1_b"])
    fc2_w, fc2_b = f(inputs["fc2_w"]), f(inputs["fc2_b"])

    wqkv = np.ascontiguousarray(
        (qkv_w * ln1_g[None, :] * S).T.reshape(KS, P, 3 * C).transpose(1, 0, 2)
    ).astype(NP8)
    bqkv_full = S * (qkv_w @ ln1_b)                # [2304], S-scaled
    bqkv = np.ascontiguousarray(bqkv_full[:2 * C].reshape(12, P).T)
    bv = np.ascontiguousarray(bqkv_full[2 * C:])
    wproj = np.ascontiguousarray(
        (proj_w * S).T.reshape(KS, P, C).transpose(1, 0, 2)).astype(NP8)
    w1 = np.ascontiguousarray(
        (fc1_w * ln2_g[None, :] * S).T.reshape(KS, P, HID).transpose(1, 0, 2))
    w1h, w1l = _hilo(w1)
    b1 = np.ascontiguousarray((fc1_b + fc1_w @ ln2_b).reshape(HS, P).T)
    w2 = np.ascontiguousarray(
        (fc2_w * S).T.reshape(HS, P, C).transpose(1, 0, 2))
    w2h, w2l = _hilo(w2)

    shared = {
        "wqkv": wqkv, "bqkv": bqkv, "bv": bv,
        "wproj": wproj, "bproj": S * proj_b,
        "w1h": w1h, "w1l": w1l, "b1": b1,
        "w2h": w2h, "w2l": w2l, "b2": S * fc2_b,
    }
    in_maps = []
    for c in range(8):
        m = dict(shared)
        m["x"] = np.ascontiguousarray(
            S * x[c * B_PER_CORE:(c + 1) * B_PER_CORE].reshape(T, C))
        in_maps.append(m)
    return in_maps


def kernel(**inputs):
    global _CACHED_NC
    b1_host = (np.asarray(inputs["fc1_b"], np.float32)
               + np.asarray(inputs["fc1_w"], np.float32)
               @ np.asarray(inputs["ln2_b"], np.float32))
    b1_zero = bool(np.all(b1_host == 0.0))
    bqkv_host = (np.asarray(inputs["qkv_w"], np.float32)
                 @ np.asarray(inputs["ln1_b"], np.float32))
    bv_zero = bool(np.all(bqkv_host[2 * C:] == 0.0))
    bqk_zero = bool(np.all(bqkv_host[:2 * C] == 0.0))
    bproj_zero = bool(np.all(np.asarray(inputs["proj_b"]) == 0.0))
    b2_zero = bool(np.all(np.asarray(inputs["fc2_b"]) == 0.0))
    key = (b1_zero, bv_zero, bproj_zero, b2_zero, bqk_zero)
    if _CACHED_NC is None or getattr(_CACHED_NC, "_spec", None) != key:
        _CACHED_NC = _build(b1_zero=b1_zero, bv_zero=bv_zero,
                            bproj_zero=bproj_zero, b2_zero=b2_zero,
                            bqk_zero=bqk_zero)
        _CACHED_NC._spec = key
    nc = _CACHED_NC
    in_maps = _prep_host(inputs)
    trace = os.environ.get("TRN_KERNEL_TRACE", "0") == "1"
    res = run_bass_kernel_spmd(nc, in_maps, core_ids=list(range(8)),
                               trace=trace)
    if trace and res.exec_time_ns is not None:
        print(f"HW exec time: {res.exec_time_ns} ns")
        print(f"mean exec time: {res.mean_exec_time_ns} ns")
        if res.instructions_and_trace is not None:
            print(f"trace: {res.instructions_and_trace[1]}")
    out = np.stack([
        res.results[c]["out"].reshape(B_PER_CORE, SEQ, C) for c in range(8)
    ]).reshape(16, SEQ, C)
    return (out / S).astype(np.float32)
